# revision 1
# baseline (speedup 1.0000x reference)
"""Octahedral SHT on 8 NeuronCores (Bass/Tile).

Strategy: shard the 192 latitude rings across 8 cores (24 rings each). Each
ring's ragged DFT (nlon in 20..400) is cut into K=128 chunks, zero-padded;
the ring assignment is engineered so every core gets exactly 51 chunks
-> one uniform SPMD program. The per-ring Legendre weights are replicated
per chunk, which folds the intra-ring chunk reduction into phase 2.
Each core computes a partial [l, m, bev] coefficient tensor over its own
rings; the host sums the 8 partials and assembles the complex output.

Precision: fp32 matmuls on the PE are 4x slower, so each fp32 operand is
split hi/lo into two fp16 tensors (x = hi + lo, |lo| <= 2^-11 |x|).
fp16*fp16 products are exact in the fp32 PSUM accumulator, so accumulating
MMs (hi*hi + hi*lo + lo*hi) reproduce the fp32 product to ~2^-22.

Phase 1 (per chunk c): G[c][m, (r,bev)] = E2[c].T @ x[c]  (PE, 6 MMs N=128,
  psum partition dim = m so the flatten yields m-major G' rows)
Flatten: G'[row c] <- [m, (hi|lo)] bounced through DRAM: SBUF->DRAM writes
  run at full HBM rate (the direct SBUF->SBUF gather is wall-limited by
  single-partition write bandwidth), then G' loads back in m-quarters whose
  SBUF writes spread across all 51 partitions, pipelined with phase-2 MMs.
  Row layout m*512 + h*256 + r*128 + bev.
Phase 2 (per m): out[l, (r,bev)] = 3 MMs (K=51):
  pw_hi @ G'hi (start) ; pw_hi @ G'lo ; pw_lo @ G'hi (stop)
"""
import numpy as np

NLAT, LMAX, MMAX = 192, 128, 128
B, V = 2, 64
BF = B * V            # 128 fused batch (b*64+v)
NCORES = 8
CHUNK = 128
NCH = 51              # chunks per core
RINGS_PER_CORE = 24
MAX_NLON = 400
NPTS = 40320
GB = [0, 4, 7, 10, 13, 17, 20, 23, 26, 30, 33, 36, 39, 43, 46, 49, NCH]
MG = 2                      # m's per psum tile (1 PSUM bank)
OG = 8                      # m's per out DMA group
PWG = 16                    # m's per pw/G' load group


def _octa_nlon():
    half = NLAT // 2
    north = np.array([4 * (i + 1) + 16 for i in range(half)], dtype=np.int64)
    return np.concatenate([north, north[::-1]])


def _ring_assignment():
    nlon = _octa_nlon()
    v = np.ceil(nlon / CHUNK).astype(int)
    cores = [[] for _ in range(NCORES)]
    for cls in (1, 2, 3, 4):
        ids = np.where(v == cls)[0]
        ids = ids[np.argsort(-nlon[ids], kind="stable")]
        fwd = True
        for start in range(0, len(ids), NCORES):
            blk = ids[start:start + NCORES]
            order = range(NCORES) if fwd else range(NCORES - 1, -1, -1)
            for c, rid in zip(order, blk):
                cores[c].append(int(rid))
            fwd = not fwd
    return cores, nlon


def _split16(a):
    hi = a.astype(np.float16)
    lo = (a - hi.astype(np.float32)).astype(np.float16)
    return hi, lo


def _build_core_inputs(core_rings, nlon, offs, x, E_re, E_im, PwT):
    """x: [BF, npts] f32.  Returns:
    xe  [128 j, 51 c, 768] f16  cols: [x_hi | x_lo | Ehi_re | Ehi_im | Elo_re | Elo_im]
    pw  [51, 128 m, 256] f16    cols: [pw_hi 0:128 | pw_lo 128:256]
    """
    xpad = np.zeros((NCH, CHUNK, BF), np.float32)
    E2 = np.zeros((NCH, CHUNK, 2 * MMAX), np.float32)
    Pw2 = np.zeros((MMAX, NCH, LMAX), np.float32)
    c = 0
    for r in core_rings:
        nl = int(nlon[r])
        o = int(offs[r])
        for j0 in range(0, nl, CHUNK):
            jlen = min(CHUNK, nl - j0)
            xpad[c, :jlen, :] = x[:, o + j0:o + j0 + jlen].T
            elen = min(CHUNK, MAX_NLON - j0)
            if elen > 0:
                E2[c, :elen, 0:MMAX] = E_re[r, j0:j0 + elen, :]
                E2[c, :elen, MMAX:] = E_im[r, j0:j0 + elen, :]
            Pw2[:, c, :] = PwT[:, r, :]
            c += 1
    assert c == NCH
    xh, xl = _split16(xpad)
    eh, el = _split16(E2)
    # eh/el cols: [re 0:128 | im 128:256]
    xe = np.concatenate([xh, xl, eh[:, :, 0:128], eh[:, :, 128:256],
                         el[:, :, 0:128], el[:, :, 128:256]], axis=2)
    xe = np.ascontiguousarray(xe.transpose(1, 0, 2))  # [128 j, 51 c, 768]

    ph, pl = _split16(Pw2)                           # [m, c, l] each
    pw = np.zeros((NCH, MMAX, 2 * LMAX), np.float16)
    pw[:, :, 0:128] = ph.transpose(1, 0, 2)          # pw_hi
    pw[:, :, 128:256] = pl.transpose(1, 0, 2)        # pw_lo
    return xe, pw


def _build_bass():
    import concourse.bass as bass
    import concourse.mybir as mybir
    from concourse import bacc, tile

    dt = mybir.dt
    nc = bacc.Bacc()

    xe_d = nc.dram_tensor("xe", [CHUNK, NCH, 768], dt.float16,
                          kind="ExternalInput")
    pw_d = nc.dram_tensor("pw", [NCH, MMAX, 2 * LMAX], dt.float16,
                          kind="ExternalInput")
    outp_d = nc.dram_tensor("outp", [LMAX, MMAX, 2 * BF], dt.float32,
                            kind="ExternalOutput")
    gdram = nc.dram_tensor("gdram", [NCH, MMAX * 512], dt.float16)

    with tile.TileContext(nc) as tc:
        with (
            tc.tile_pool(name="xs", bufs=4) as xs_pool,
            tc.tile_pool(name="gt", bufs=4) as gt_pool,
            tc.tile_pool(name="gs", bufs=4) as gs_pool,
            tc.tile_pool(name="pws", bufs=4) as pw_pool,
            tc.tile_pool(name="os", bufs=2) as os_pool,
            tc.tile_pool(name="ps1", bufs=2, space="PSUM") as ps1,
            tc.tile_pool(name="ps2", bufs=3, space="PSUM") as ps2,
        ):
            # ---- phase 1: 51 chunks x 6 accumulating MMs ----
            NG = len(GB) - 1
            xg = {}
            for g in range(NG):
                n = GB[g + 1] - GB[g]
                t = xs_pool.tile([CHUNK, n * 768], dt.float16, tag="xg")
                # split by partition halves: 2 parallel DMA engines per group
                nc.sync.dma_start(out=t[0:64, :],
                                  in_=xe_d[0:64, GB[g]:GB[g + 1], :])
                nc.scalar.dma_start(out=t[64:128, :],
                                    in_=xe_d[64:128, GB[g]:GB[g + 1], :])
                xg[g] = t

            for c in range(NCH):
                g = next(i for i in range(NG) if GB[i] <= c < GB[i + 1])
                off = (c - GB[g]) * 768
                xe = xg[g]
                xh = xe[:, off + 0:off + 128]
                xl = xe[:, off + 128:off + 256]
                ehr = xe[:, off + 256:off + 384]
                ehi = xe[:, off + 384:off + 512]
                elr = xe[:, off + 512:off + 640]
                eli = xe[:, off + 640:off + 768]
                # psum [m, (re_bev | im_bev)]; 6 MMs, lhsT = E slices
                g_ps = ps1.tile([MMAX, 2 * BF], dt.float32, tag="g")
                re = g_ps[:, 0:128]
                im = g_ps[:, 128:256]
                nc.tensor.matmul(re, ehr, xh, start=True, stop=False)
                nc.tensor.matmul(re, ehr, xl, start=False, stop=False)
                nc.tensor.matmul(re, elr, xh, start=False, stop=True)
                nc.tensor.matmul(im, ehi, xh, start=True, stop=False)
                nc.tensor.matmul(im, ehi, xl, start=False, stop=False)
                nc.tensor.matmul(im, eli, xh, start=False, stop=True)
                # evacuate PSUM, splitting fp32 -> fp16 hi (ACT) + lo (DVE)
                g_hl = gt_pool.tile([MMAX, 512], dt.float16, tag="ghl")
                nc.scalar.copy(g_hl[:, 0:256], g_ps[:])
                nc.vector.tensor_sub(g_hl[:, 256:512], g_ps[:], g_hl[:, 0:256])
                # flatten to DRAM (full-rate HBM write, 1KB runs)
                nc.gpsimd.dma_start(out=gdram[c], in_=g_hl[:])

            # ---- phase 2: 128 m x 3 accumulating MMs ----
            for mg in range(0, MMAX, PWG):
                pwt = pw_pool.tile([NCH, PWG * 256], dt.float16, tag="pw")
                nc.sync.dma_start(out=pwt[:], in_=pw_d[:, mg:mg + PWG, :])
                gsb = gs_pool.tile([NCH, PWG * 512], dt.float16, tag="gq")
                # two parallel loads (8KB runs each); the first groups ride
                # the scalar queue, which is idle by the end of phase 1
                half = PWG * 256
                if mg < 32:
                    ea, eb = nc.scalar, nc.scalar
                else:
                    ea, eb = nc.sync, nc.gpsimd
                ea.dma_start(
                    out=gsb[:, 0:half],
                    in_=gdram[:, mg * 512:mg * 512 + half])
                eb.dma_start(
                    out=gsb[:, half:2 * half],
                    in_=gdram[:, mg * 512 + half:(mg + PWG) * 512])
                for m8 in range(mg, mg + PWG, OG):
                    lb = 96 if m8 >= 96 else (64 if m8 >= 64 else 0)
                    o_sb = os_pool.tile([LMAX, OG * 256], dt.float32, tag="ot")
                    for m4 in range(m8, m8 + OG, 2):
                        # two m's in separate psum banks, MMs interleaved so
                        # consecutive PE MMs hit independent accum chains
                        o_ps0 = ps2.tile([LMAX, 256], dt.float32, tag="o0")
                        o_ps1 = ps2.tile([LMAX, 256], dt.float32, tag="o1")
                        ops = [o_ps0, o_ps1]
                        for term in range(3):
                            for i, m in enumerate((m4, m4 + 1)):
                                mo = (m - mg) * 256
                                ml = (m - mg) * 512
                                pa = pwt[:, mo + 0:mo + 128]
                                pb = pwt[:, mo + 128:mo + 256]
                                lhs, rhs = (
                                    (pa, gsb[:, ml:ml + 256]),
                                    (pa, gsb[:, ml + 256:ml + 512]),
                                    (pb, gsb[:, ml:ml + 256]),
                                )[term]
                                nc.tensor.matmul(ops[i][:], lhs, rhs,
                                                 start=(term == 0),
                                                 stop=(term == 2))
                        # coeffs[l < m] == 0 structurally; evacuate only
                        # rows l >= lb (legal partition bases {0,64,96};
                        # output buffer is pre-zeroed)
                        for i in range(2):
                            oo = (m4 + i - m8) * 256
                            if (m4 // 2 + i) % 2 == 0:
                                nc.vector.tensor_copy(o_sb[lb:, oo:oo + 256],
                                                      ops[i][lb:, :])
                            else:
                                nc.scalar.copy(o_sb[lb:, oo:oo + 256],
                                               ops[i][lb:, :])
                    eng = nc.sync if (m8 // OG) % 2 == 0 else nc.gpsimd
                    eng.dma_start(out=outp_d[lb:, m8:m8 + OG, :],
                                  in_=o_sb[lb:, :])

    nc.compile()
    return nc


_CACHE = {}


def _get_compiled():
    if "nc" not in _CACHE:
        _CACHE["nc"] = _build_bass()
    return _CACHE["nc"]


def kernel(data, Pw, E_re, E_im, pad_idx):
    from concourse import bass_utils

    data = np.asarray(data)
    Pw = np.asarray(Pw, dtype=np.float32)
    E_re = np.asarray(E_re, dtype=np.float32)
    E_im = np.asarray(E_im, dtype=np.float32)

    cores, nlon = _ring_assignment()
    offs = np.concatenate([[0], np.cumsum(nlon)[:-1]])
    # 'b e p v -> (b e v) p'
    x = np.ascontiguousarray(
        np.transpose(data, (0, 1, 3, 2)).reshape(BF, NPTS).astype(np.float32))
    PwT = np.ascontiguousarray(np.transpose(Pw, (1, 2, 0)))  # [m, n, l]

    in_maps = []
    for c in range(NCORES):
        xe, pw = _build_core_inputs(cores[c], nlon, offs, x, E_re, E_im, PwT)
        in_maps.append({"xe": xe, "pw": pw})

    nc = _get_compiled()
    res = bass_utils.run_bass_kernel_spmd(nc, in_maps, list(range(NCORES)))
    _CACHE["last_results"] = res

    total = np.zeros((LMAX, MMAX, 2 * BF), np.float64)
    for r in res.results:
        total += r["outp"].astype(np.float64)
    total = total.astype(np.float32).reshape(LMAX, MMAX, 2, BF)
    cc = total[:, :, 0, :] + 1j * total[:, :, 1, :]  # [l, m, bev]
    cc = cc.reshape(LMAX, MMAX, B, V)
    out = np.transpose(cc, (2, 0, 1, 3))[:, None]    # [b, 1, l, m, v]
    return out.astype(np.complex64)



# revision 7
# speedup vs baseline: 2.8907x; 2.8907x over previous
"""Octahedral SHT on 8 NeuronCores (Bass/Tile), quarter-folded fp16 design.

Strategy: shard the 192 latitude rings across 8 cores (24 each). The ragged
per-ring DFT is quarter-folded on the host using the cosine/sine symmetries
j<->n-j and j<->n/2-j: the folded x vectors (we/wo/ze/zo, one per m-parity x
re/im quadrant) have n/4+-1 rows, so every ring fits a single K<=101 matmul
chunk and the E matrix shrinks 4x. Tolerance is 2e-2, so all operands are
plain fp16 (measured pipeline error ~4e-4) - no hi/lo splitting.

Phase 1 (per ring): 4 MMs (re/im x even/odd m) -> psum [128 m~, 256 re|im],
rows 0:64 = even m, 64:128 = odd m. Evacuate f32->f16 (ACT/DVE alternate),
bounce through DRAM to transpose ring-index onto partitions.

Phase 2 (per m): out[bev, l] = G'[r, bev].T @ pw[r, l] with exact triangular
l-range (coeffs vanish for l < m). m's are processed in pairs (m~, 127-m~)
so each psum bank holds exactly 2*l(a)+2*l(b) = 258 f32 columns; one strided
evac per 2-bank tile, fp16 triangular output, host sums 8 partials.

Rings are assigned to (core, slot) by sorted length rank: slot s holds rank
8s..8s+7, one per core, so a per-slot staircase row count R_S[s]=101-4s is
SPMD-uniform and cuts the zero-padding DMA ~45%.
"""
import numpy as np

NLAT, LMAX, MMAX = 192, 128, 128
B, V = 2, 64
BF = B * V            # 128 fused batch (b*64+v)
NCORES = 8
SLOTS = 24            # rings per core
JP = 104              # xef partition dim (>= max folded rows 101)
NPTS = 40320
PWCOLS = 129 * 64     # 8256: pair (a, 127-a) always has l_a + l_b = 129
OUTCOLS = 516 * 32    # 16512


def _octa_nlon():
    half = NLAT // 2
    north = np.array([4 * (i + 1) + 16 for i in range(half)], dtype=np.int64)
    return np.concatenate([north, north[::-1]])


def _plan():
    nlon = _octa_nlon()
    order = np.argsort(-nlon, kind="stable")          # ring ids, length desc
    r_s = [int(nlon[order[8 * s]]) // 4 + 1 for s in range(SLOTS)]
    pairs = [(16 * bp + i, 127 - (16 * bp + i))
             for bp in range(4) for i in range(16)]   # m~ pairs, bp-major
    return nlon, order, r_s, pairs


def _true_m(mt):
    return 2 * mt if mt < 64 else 2 * (mt - 64) + 1


def _fold_ring(xr, n):
    """xr [BF, n] f32 -> (we, wo, ze, zo) with q+1, q, q-1, q rows (q=n//4)."""
    h, q = n // 2, n // 4
    u = np.empty((BF, h + 1), np.float32)
    u[:, 0] = xr[:, 0]
    u[:, h] = xr[:, h]
    u[:, 1:h] = xr[:, 1:h] + xr[:, :h:-1]
    v = xr[:, 1:h] - xr[:, :h:-1]                      # j=1..h-1 at col j-1
    jj = np.arange(1, q)
    we = np.empty((BF, q + 1), np.float32)
    we[:, 0] = u[:, 0] + u[:, h]
    we[:, q] = u[:, q]
    we[:, jj] = u[:, jj] + u[:, h - jj]
    wo = np.empty((BF, q), np.float32)
    wo[:, 0] = u[:, 0] - u[:, h]
    wo[:, jj] = u[:, jj] - u[:, h - jj]
    ze = v[:, jj - 1] - v[:, h - jj - 1]               # [BF, q-1]
    zo = np.empty((BF, q), np.float32)
    zo[:, jj - 1] = v[:, jj - 1] + v[:, h - jj - 1]
    zo[:, q - 1] = v[:, q - 1]
    return we, wo, ze, zo


def _build_core_inputs(c, nlon, order, r_s, pairs, offs, x, E_re, E_im, Pw):
    xef = np.zeros((JP, SLOTS, 768), np.float16)
    pw = np.zeros((SLOTS, PWCOLS), np.float16)
    for s in range(SLOTS):
        gid = int(order[8 * s + c])
        n = int(nlon[gid]); q = n // 4; o = int(offs[gid])
        we, wo, ze, zo = _fold_ring(x[:, o:o + n], n)
        xef[0:q + 1, s, 0:128] = we.T
        xef[0:q,     s, 128:256] = wo.T
        xef[0:q - 1, s, 256:384] = ze.T
        xef[0:q,     s, 384:512] = zo.T
        xef[0:q + 1, s, 512:576] = E_re[gid, 0:q + 1, 0::2]
        xef[0:q,     s, 576:640] = E_re[gid, 0:q, 1::2]
        xef[0:q - 1, s, 640:704] = E_im[gid, 1:q, 0::2]
        xef[0:q,     s, 704:768] = E_im[gid, 1:q + 1, 1::2]
        for p, (a, b) in enumerate(pairs):
            ma, mb = _true_m(a), _true_m(b)
            la = 128 - ma
            pw[s, 129 * p:129 * p + la] = Pw[ma:, ma, gid]
            pw[s, 129 * p + la:129 * (p + 1)] = Pw[mb:, mb, gid]
    return {"xef": xef, "pw": pw}


def _build_bass(r_s, pairs):
    import concourse.bass as bass
    import concourse.mybir as mybir
    from concourse import bacc, tile

    dt = mybir.dt
    nc = bacc.Bacc()

    xef_d = nc.dram_tensor("xef", [JP, SLOTS, 768], dt.float16,
                           kind="ExternalInput")
    pw_d = nc.dram_tensor("pw", [SLOTS, PWCOLS], dt.float16,
                          kind="ExternalInput")
    outp_d = nc.dram_tensor("outp", [BF, OUTCOLS], dt.float16,
                            kind="ExternalOutput")
    gdram = nc.dram_tensor("gdram", [SLOTS, 128 * 256], dt.float16)

    with tile.TileContext(nc) as tc:
        with (
            tc.tile_pool(name="xs", bufs=1) as xs_pool,
            tc.tile_pool(name="pws", bufs=1) as pw_pool,
            tc.tile_pool(name="g1", bufs=4) as g1_pool,
            tc.tile_pool(name="gs", bufs=4) as gs_pool,
            tc.tile_pool(name="os", bufs=2) as os_pool,
            tc.tile_pool(name="ps2", bufs=3, space="PSUM") as ps2,
            tc.tile_pool(name="ps1", bufs=2, space="PSUM") as ps1,
        ):
            QS = [nc.sync, nc.gpsimd, nc.scalar]

            xef_sb = xs_pool.tile([JP, SLOTS, 768], dt.float16)
            for s in range(SLOTS):
                QS[s % 3].dma_start(out=xef_sb[0:r_s[s], s, :],
                                    in_=xef_d[0:r_s[s], s, :])
            pw_sb = pw_pool.tile([SLOTS, PWCOLS], dt.float16)
            nc.gpsimd.dma_start(out=pw_sb[:], in_=pw_d[:])

            # ---- phase 1: 24 rings x 4 quadrant MMs ----
            for s in range(SLOTS):
                K = r_s[s]
                g_ps = ps1.tile([128, 256], dt.float32, tag="g")
                e0 = xef_sb[0:K, s, 512:576]
                e1 = xef_sb[0:K, s, 576:640]
                e2 = xef_sb[0:K, s, 640:704]
                e3 = xef_sb[0:K, s, 704:768]
                nc.tensor.matmul(g_ps[0:64, 0:128], e0,
                                 xef_sb[0:K, s, 0:128])
                nc.tensor.matmul(g_ps[64:128, 0:128], e1,
                                 xef_sb[0:K, s, 128:256])
                nc.tensor.matmul(g_ps[0:64, 128:256], e2,
                                 xef_sb[0:K, s, 256:384])
                nc.tensor.matmul(g_ps[64:128, 128:256], e3,
                                 xef_sb[0:K, s, 384:512])
                g_sb = g1_pool.tile([128, 256], dt.float16, tag="ghl")
                nc.vector.tensor_copy(g_sb[:], g_ps[:])
                QS[s % 3].dma_start(out=gdram[s], in_=g_sb[:])

            # ---- phase 2: 64 m~ pairs, exact triangular ----
            o_sb = None
            for bp in range(4):
                glo = gs_pool.tile([SLOTS, 4096], dt.float16, tag="glo")
                QS[(2 * bp) % 3].dma_start(
                    out=glo[:], in_=gdram[:, bp * 4096:(bp + 1) * 4096])
                ghi = gs_pool.tile([SLOTS, 4096], dt.float16, tag="ghi")
                QS[(2 * bp + 1) % 3].dma_start(
                    out=ghi[:], in_=gdram[:, (7 - bp) * 4096:(8 - bp) * 4096])
                for tt in range(8):
                    t = 8 * bp + tt
                    o_ps = ps2.tile([128, 2, 512], dt.float32, tag="o")
                    for b2 in range(2):
                        p = 2 * t + b2
                        a, _ = pairs[p]
                        i2 = a - 16 * bp
                        la = 128 - 2 * a
                        lb = 129 - la
                        po = 129 * p
                        nc.tensor.matmul(
                            o_ps[:, b2, 0:la],
                            glo[:, i2 * 256:i2 * 256 + 128],
                            pw_sb[:, po:po + la])
                        nc.tensor.matmul(
                            o_ps[:, b2, la:2 * la],
                            glo[:, i2 * 256 + 128:i2 * 256 + 256],
                            pw_sb[:, po:po + la])
                        nc.tensor.matmul(
                            o_ps[:, b2, 2 * la:2 * la + lb],
                            ghi[:, (15 - i2) * 256:(15 - i2) * 256 + 128],
                            pw_sb[:, po + la:po + 129])
                        nc.tensor.matmul(
                            o_ps[:, b2, 2 * la + lb:258],
                            ghi[:, (15 - i2) * 256 + 128:(15 - i2) * 256 + 256],
                            pw_sb[:, po + la:po + 129])
                    if t % 4 == 0:
                        o_sb = os_pool.tile([128, 2064], dt.float16, tag="ot")
                    dst = o_sb[:, (t % 4) * 516:(t % 4 + 1) * 516]
                    if t % 2 == 0:
                        nc.scalar.copy(dst, o_ps[:, :, 0:258])
                    else:
                        nc.vector.tensor_copy(dst, o_ps[:, :, 0:258])
                    if t % 4 == 3:
                        QS[(t // 4) % 3].dma_start(
                            out=outp_d[:, (t - 3) * 516:(t + 1) * 516],
                            in_=o_sb[:])

    nc.compile()
    return nc


_CACHE = {}


def _get_compiled(r_s, pairs):
    if "nc" not in _CACHE:
        _CACHE["nc"] = _build_bass(r_s, pairs)
    return _CACHE["nc"]


def kernel(data, Pw, E_re, E_im, pad_idx):
    from concourse import bass_utils

    data = np.asarray(data)
    Pw = np.asarray(Pw, dtype=np.float32)
    E_re = np.asarray(E_re, dtype=np.float32)
    E_im = np.asarray(E_im, dtype=np.float32)

    nlon, order, r_s, pairs = _plan()
    offs = np.concatenate([[0], np.cumsum(nlon)[:-1]])
    # 'b e p v -> (b e v) p'
    x = np.ascontiguousarray(
        np.transpose(data, (0, 1, 3, 2)).reshape(BF, NPTS).astype(np.float32))

    in_maps = [
        _build_core_inputs(c, nlon, order, r_s, pairs, offs, x, E_re, E_im, Pw)
        for c in range(NCORES)
    ]

    nc = _get_compiled(r_s, pairs)
    res = bass_utils.run_bass_kernel_spmd(nc, in_maps, list(range(NCORES)))
    _CACHE["last_results"] = res

    total = np.zeros((BF, OUTCOLS), np.float64)
    for r in res.results:
        total += r["outp"].astype(np.float64)

    coeffs = np.zeros((LMAX, MMAX, BF), np.complex128)
    for t in range(32):
        for b2 in range(2):
            p = 2 * t + b2
            a, b = pairs[p]
            ma, mb = _true_m(a), _true_m(b)
            la, lb = 128 - ma, 128 - mb
            base = 516 * t + 258 * b2
            re_a = total[:, base:base + la]
            im_a = total[:, base + la:base + 2 * la]
            re_b = total[:, base + 2 * la:base + 2 * la + lb]
            im_b = total[:, base + 2 * la + lb:base + 258]
            coeffs[ma:, ma, :] = (re_a + 1j * im_a).T
            coeffs[mb:, mb, :] = (re_b + 1j * im_b).T
    cc = coeffs.reshape(LMAX, MMAX, B, V)
    out = np.transpose(cc, (2, 0, 1, 3))[:, None]    # [b, 1, l, m, v]
    return out.astype(np.complex64)
